# revision 18
# baseline (speedup 1.0000x reference)
"""ColBERT loss kernel for Trainium2 (8 NeuronCores, SPMD).

Shapes (hardcoded per problem spec):
  query_embeddings (64, 64, 128) f32, doc_embeddings (64, 512, 128) f32,
  query_mask (64, 64) bool, doc_mask (64, 512) bool -> scalar f32 loss.

Strategy:
  - Shard the 64 queries over 8 cores (8 queries = 4 query-pairs per core);
    docs are replicated (8 MB bf16, resident in SBUF).
  - Per core: 256 bf16 matmuls [K=128, M=128(2 queries x 64 tokens), N=512
    (1 doc's tokens)] -> PSUM; the max over doc tokens is drained from PSUM
    by two balanced routes (DVE tensor_reduce directly from PSUM, and
    ACT copy->SBUF bf16 + DVE tensor_scalar(max)+accum_out at 4x mode).
  - Sum over query tokens (with query_mask and 1/temperature folded in) is a
    tiny matmul against a host-built [128, 2] constant.
  - Each core outputs its full score rows [8, 64]; the final log-softmax +
    mean over the diagonal runs on host (4096 elements).
"""

import sys
import types

import numpy as np
import ml_dtypes


def _install_ntff_shim():
    """bass_utils unconditionally imports antenv.axon_hooks when tracing is
    requested (e.g. BASS_TRACE=1 in the environment); the module is absent in
    this image. Register a null hook so the import succeeds and tracing
    degrades gracefully instead of crashing the run."""
    if 'antenv.axon_hooks' in sys.modules:
        return
    try:
        import antenv
    except ImportError:
        return
    mod = types.ModuleType('antenv.axon_hooks')
    mod._hook = None

    def set_axon_ntff_profile_hook(h):
        mod._hook = h

    def get_axon_ntff_profile_hook():
        return mod._hook

    mod.set_axon_ntff_profile_hook = set_axon_ntff_profile_hook
    mod.get_axon_ntff_profile_hook = get_axon_ntff_profile_hook
    sys.modules['antenv.axon_hooks'] = mod
    antenv.axon_hooks = mod


_install_ntff_shim()

import concourse.bacc as bacc
import concourse.mybir as mybir
import concourse.tile as tile
from concourse.bass_utils import run_bass_kernel_spmd

F32 = mybir.dt.float32
F16 = mybir.dt.float16
F16_NP = np.float16

N_CORES = 8
BQ, SQ, D = 64, 64, 128
BD, SD = 64, 512
Q_PER_CORE = BQ // N_CORES          # 8
PAIRS = Q_PER_CORE // 2             # 4
GROUPS = 16                         # doc chunks of 4 docs
DOCS_PER_GROUP = BD // GROUPS       # 4
INV_TEMP = 50.0                     # 1 / 0.02
NEG_BIG = -3.0e38

# Per-(group,pair)-unit drain-route split, balanced so DVE and ACT finish
# together (see notes.md): route A = DVE reduce direct from PSUM (PSUM-port
# floor, ~2.28us/unit), route B = ACT copy->SBUF bf16 (~2.0us/unit) + DVE
# tensor_tensor max tree at 2x_1p over PAIRS of B units (~2.9us/pair).
# A units are spread finely so DVE(A) and ACT(B) drains of the two PSUM
# slots overlap instead of serializing.
ROUTE_A_MOD16 = {7, 12}
# First units route A so DVE has work before the first ACT copies land; last
# units route A so the kernel tail is a short reduce, not a copy+tree chain.
ROUTE_A_EXTRA = {0, 1, 61, 63}

_CACHE = {}


def _build_nc():
    nc = bacc.Bacc("TRN2", target_bir_lowering=False, debug=False,
                   num_devices=N_CORES)
    qT = nc.dram_tensor("qT", [PAIRS, 128, 128], F16, kind="ExternalInput").ap()
    dT = nc.dram_tensor("dT", [128, BD * SD], F16, kind="ExternalInput").ap()
    ones = nc.dram_tensor("ones", [128, Q_PER_CORE], F32, kind="ExternalInput").ap()
    # [2, PAIRS*BD]: row = pair member (query 2p / 2p+1), cols pair-major
    scores_out = nc.dram_tensor("scores", [2, PAIRS * BD], F32,
                                kind="ExternalOutput").ap()

    with tile.TileContext(nc) as tc:
        with (
            tc.tile_pool(name="qpool", bufs=1) as qpool,
            tc.tile_pool(name="docs", bufs=1) as dpool,
            tc.tile_pool(name="psum", bufs=2, space="PSUM") as pspool,
            tc.tile_pool(name="bcopy", bufs=3) as bpool,
            tc.tile_pool(name="scratch", bufs=2) as spool,
            tc.tile_pool(name="maxs", bufs=1) as mpool,
            tc.tile_pool(name="small", bufs=1) as smallpool,
        ):
            maxs = mpool.tile([128, PAIRS * BD], F32)

            # First doc chunk + queries first so matmuls can start ASAP.
            dtiles = [dpool.tile([128, DOCS_PER_GROUP * SD], F16,
                                 name=f"d{g}", tag=f"d{g}")
                      for g in range(GROUPS)]
            nc.sync.dma_start(dtiles[0][:], dT[:, 0:DOCS_PER_GROUP * SD])
            qtile = qpool.tile([128, PAIRS * 128], F16)
            nc.sync.dma_start(
                qtile[:].rearrange("q (p t) -> q p t", p=PAIRS),
                qT[:].rearrange("p q t -> q p t"))
            for g in range(1, GROUPS):
                nc.sync.dma_start(
                    dtiles[g][:],
                    dT[:, g * DOCS_PER_GROUP * SD:(g + 1) * DOCS_PER_GROUP * SD])
            # only needed by the final score matmuls
            otile = smallpool.tile([128, Q_PER_CORE], F32, tag="ones")
            nc.sync.dma_start(otile[:], ones[:])

            # maxs viewed as [128, pair, doc] for strided pair-tree outputs
            maxs_v = maxs[:].rearrange("q (p b) -> q p b", b=BD)

            def tt_max(out, in0, in1):
                nc.vector.tensor_tensor(out, in0, in1, op=mybir.AluOpType.max)

            def halves(ap, n):
                v = ap.rearrange("q (d h n) -> q d h n", h=2, n=n)
                return v[:, :, 0], v[:, :, 1]

            w = DOCS_PER_GROUP * SD
            pend = None                    # (sb_tile, (g, p)) of pending B half

            def emit_tree(sb, u0, u1):
                nd = 2 * DOCS_PER_GROUP    # 8 docs in the pair
                t1 = spool.tile([128, nd * 256], F16, name="t1", tag="t1")
                tt_max(t1[:].rearrange("q (d n) -> q d n", n=256),
                       *halves(sb[:], 256))
                t2 = spool.tile([128, nd * 128], F16, name="t2", tag="t2")
                tt_max(t2[:].rearrange("q (d n) -> q d n", n=128),
                       *halves(t1[:], 128))
                t3 = spool.tile([128, nd * 64], F16, name="t3", tag="t3")
                tt_max(t3[:].rearrange("q (d n) -> q d n", n=64),
                       *halves(t2[:], 64))
                (g0, p0), (g1, p1) = u0, u1
                if g0 == g1 and p1 == p0 + 1:
                    nc.vector.tensor_reduce(
                        maxs_v[:, p0:p0 + 2,
                               g0 * DOCS_PER_GROUP:(g0 + 1) * DOCS_PER_GROUP],
                        t3[:].rearrange("q (u d n) -> q u d n", u=2, n=64),
                        axis=mybir.AxisListType.X, op=mybir.AluOpType.max)
                else:
                    for h, (gh, ph) in enumerate(
                            ((g0, p0), (g1, p1))):
                        nc.vector.tensor_reduce(
                            maxs_v[:, ph:ph + 1,
                                   gh * DOCS_PER_GROUP:(gh + 1) * DOCS_PER_GROUP],
                            t3[:, h * DOCS_PER_GROUP * 64:(h + 1) * DOCS_PER_GROUP * 64]
                            .rearrange("q (d n) -> q d n", n=64),
                            axis=mybir.AxisListType.X, op=mybir.AluOpType.max)

            for g in range(GROUPS):
                for p in range(PAIRS):
                    unit = g * PAIRS + p
                    ps = pspool.tile([128, DOCS_PER_GROUP * SD], F32,
                                     name="ps", tag="ps")
                    for j in range(DOCS_PER_GROUP):
                        nc.tensor.matmul(
                            ps[:, j * SD:(j + 1) * SD],
                            lhsT=qtile[:, p * 128:(p + 1) * 128],
                            rhs=dtiles[g][:, j * SD:(j + 1) * SD],
                            start=True, stop=True,
                        )
                    col = p * BD + g * DOCS_PER_GROUP
                    if unit % 16 in ROUTE_A_MOD16 or unit in ROUTE_A_EXTRA:
                        nc.vector.tensor_reduce(
                            maxs[:, col:col + DOCS_PER_GROUP],
                            ps[:].rearrange("q (d n) -> q d n", n=SD),
                            axis=mybir.AxisListType.X,
                            op=mybir.AluOpType.max,
                        )
                        continue
                    # Route B: ACT copies this unit's 4 docs into one half of a
                    # double-wide bf16 tile; when the second half lands, one
                    # shared TT-max tree (2x_1p mode) drains all 8 docs.
                    if pend is None:
                        sb = bpool.tile([128, 2 * w], F16, name="b", tag="b")
                        nc.scalar.copy(sb[:, 0:w], ps[:])
                        pend = (sb, (g, p))
                    else:
                        sb, u0 = pend
                        nc.scalar.copy(sb[:, w:2 * w], ps[:])
                        emit_tree(sb, u0, (g, p))
                        pend = None
            assert pend is None, "odd number of route-B units"

            # scores[2, 64] per pair = (INV_TEMP * qmask)-weighted column sums
            scores_sb = smallpool.tile([2, PAIRS * BD], F32, tag="ssb")
            for p in range(PAIRS):
                sc_ps = pspool.tile([128, DOCS_PER_GROUP * SD], F32, tag="ps")
                nc.tensor.matmul(
                    sc_ps[0:2, 0:BD],
                    lhsT=otile[:, 2 * p:2 * p + 2],
                    rhs=maxs[:, p * BD:(p + 1) * BD],
                    start=True, stop=True,
                )
                nc.vector.tensor_copy(scores_sb[0:2, p * BD:(p + 1) * BD],
                                      sc_ps[0:2, 0:BD])
            nc.sync.dma_start(scores_out[:], scores_sb[:])

    nc.compile()
    return nc


def _get_nc():
    if "nc" not in _CACHE:
        _CACHE["nc"] = _build_nc()
    return _CACHE["nc"]


def _compact_doc_tokens(doc, mask):
    """Reorder each doc's tokens so masked slots are replaced by duplicates of
    a valid token (max over tokens is unchanged). Exact for any doc with at
    least one valid token."""
    out = doc.copy()
    for i in range(doc.shape[0]):
        m = mask[i]
        if m.all():
            continue
        valid = np.where(m)[0]
        idx = np.where(m, np.arange(doc.shape[1]), valid[0])
        out[i] = doc[i, idx]
    return out


def _host_reference(query_embeddings, doc_embeddings, query_mask, doc_mask):
    """Exact (fp32-semantics) fallback, only used for degenerate masks."""
    q = np.asarray(query_embeddings, np.float32)
    d = np.asarray(doc_embeddings, np.float32)
    sim = np.einsum('qnd,pmd->qpnm', q, d).astype(np.float32)
    sim = np.where(np.asarray(doc_mask, bool)[None, :, None, :], sim,
                   np.float32(-1e30))
    mx = sim.max(axis=-1)
    mx = mx * np.asarray(query_mask, np.float32)[:, None, :]
    scores = mx.sum(axis=-1) / np.float32(0.02)
    return _loss_from_scores(scores)


def _loss_from_scores(scores):
    s = np.asarray(scores, np.float64)
    m = s.max(axis=-1, keepdims=True)
    lse = m[:, 0] + np.log(np.exp(s - m).sum(axis=-1))
    return np.float32(np.mean(lse - np.diagonal(s)))


def kernel(query_embeddings, doc_embeddings, query_mask, doc_mask):
    q = np.ascontiguousarray(np.asarray(query_embeddings, dtype=np.float32))
    d = np.ascontiguousarray(np.asarray(doc_embeddings, dtype=np.float32))
    qm = np.asarray(query_mask, dtype=bool)
    dm = np.asarray(doc_mask, dtype=bool)
    assert q.shape == (BQ, SQ, D) and d.shape == (BD, SD, D)

    if not dm.all():
        if not dm.any(axis=1).all():
            # A fully-masked doc makes every max -1e30; the kernel's
            # compaction trick can't represent that, fall back entirely.
            return _host_reference(q, d, qm, dm)
        d = _compact_doc_tokens(d, dm)

    # [128(D), 64*512] doc tokens, bf16
    dT = np.ascontiguousarray(d.transpose(2, 0, 1).reshape(D, BD * SD)).astype(F16_NP)

    qmf = qm.astype(np.float32) * INV_TEMP
    in_maps = []
    for c in range(N_CORES):
        qc = q[c * Q_PER_CORE:(c + 1) * Q_PER_CORE]          # [8, 64, 128]
        # [pair, D, 128 tokens] bf16
        qT = np.ascontiguousarray(
            qc.reshape(PAIRS, 2 * SQ, D).transpose(0, 2, 1)).astype(F16_NP)
        ones = np.zeros((128, Q_PER_CORE), np.float32)
        for j in range(Q_PER_CORE):
            p, mzz = j // 2, j % 2
            ones[mzz * SQ:(mzz + 1) * SQ, j] = qmf[c * Q_PER_CORE + 2 * p + mzz]
        in_maps.append({"qT": qT, "dT": dT, "ones": ones})

    nc = _get_nc()
    res = run_bass_kernel_spmd(nc, in_maps, list(range(N_CORES)))
    scores = np.concatenate(
        [res.results[c]["scores"].reshape(2, PAIRS, BD).transpose(1, 0, 2)
         .reshape(Q_PER_CORE, BD) for c in range(N_CORES)], axis=0)  # [64, 64]
    return _loss_from_scores(scores)


if __name__ == "__main__":
    rng = np.random.default_rng(0)
    inputs = {
        "query_embeddings": rng.standard_normal((BQ, SQ, D), dtype=np.float32),
        "doc_embeddings": rng.standard_normal((BD, SD, D), dtype=np.float32),
        "query_mask": np.ones((BQ, SQ), bool),
        "doc_mask": np.ones((BD, SD), bool),
    }
    out = kernel(**inputs)
    ref = _host_reference(**inputs)
    print("kernel:", out, "ref:", ref, "rel:", abs(out - ref) / abs(ref))


# revision 19
# speedup vs baseline: 1.0148x; 1.0148x over previous
"""ColBERT loss kernel for Trainium2 (8 NeuronCores, SPMD).

Shapes (hardcoded per problem spec):
  query_embeddings (64, 64, 128) f32, doc_embeddings (64, 512, 128) f32,
  query_mask (64, 64) bool, doc_mask (64, 512) bool -> scalar f32 loss.

Strategy:
  - Shard the 64 queries over 8 cores (8 queries = 4 query-pairs per core);
    docs are replicated (8 MB bf16, resident in SBUF).
  - Per core: 256 bf16 matmuls [K=128, M=128(2 queries x 64 tokens), N=512
    (1 doc's tokens)] -> PSUM; the max over doc tokens is drained from PSUM
    by two balanced routes (DVE tensor_reduce directly from PSUM, and
    ACT copy->SBUF bf16 + DVE tensor_scalar(max)+accum_out at 4x mode).
  - Sum over query tokens (with query_mask and 1/temperature folded in) is a
    tiny matmul against a host-built [128, 2] constant.
  - Each core outputs its full score rows [8, 64]; the final log-softmax +
    mean over the diagonal runs on host (4096 elements).
"""

import sys
import types

import numpy as np
import ml_dtypes


def _install_ntff_shim():
    """bass_utils unconditionally imports antenv.axon_hooks when tracing is
    requested (e.g. BASS_TRACE=1 in the environment); the module is absent in
    this image. Register a null hook so the import succeeds and tracing
    degrades gracefully instead of crashing the run."""
    if 'antenv.axon_hooks' in sys.modules:
        return
    try:
        import antenv
    except ImportError:
        return
    mod = types.ModuleType('antenv.axon_hooks')
    mod._hook = None

    def set_axon_ntff_profile_hook(h):
        mod._hook = h

    def get_axon_ntff_profile_hook():
        return mod._hook

    mod.set_axon_ntff_profile_hook = set_axon_ntff_profile_hook
    mod.get_axon_ntff_profile_hook = get_axon_ntff_profile_hook
    sys.modules['antenv.axon_hooks'] = mod
    antenv.axon_hooks = mod


_install_ntff_shim()

import concourse.bacc as bacc
import concourse.mybir as mybir
import concourse.tile as tile
from concourse.bass_utils import run_bass_kernel_spmd

F32 = mybir.dt.float32
F16 = mybir.dt.float16
F16_NP = np.float16

N_CORES = 8
BQ, SQ, D = 64, 64, 128
BD, SD = 64, 512
Q_PER_CORE = BQ // N_CORES          # 8
PAIRS = Q_PER_CORE // 2             # 4
GROUPS = 16                         # doc chunks of 4 docs
DOCS_PER_GROUP = BD // GROUPS       # 4
INV_TEMP = 50.0                     # 1 / 0.02
NEG_BIG = -3.0e38

# Per-(group,pair)-unit drain-route split, balanced so DVE and ACT finish
# together (see notes.md): route A = DVE reduce direct from PSUM (PSUM-port
# floor, ~2.28us/unit), route B = ACT copy->SBUF bf16 (~2.0us/unit) + DVE
# tensor_tensor max tree at 2x_1p over PAIRS of B units (~2.9us/pair).
# A units are spread finely so DVE(A) and ACT(B) drains of the two PSUM
# slots overlap instead of serializing.
ROUTE_A_MOD16 = {2, 7, 12}
# First units route A so DVE has work before the first ACT copies land; last
# units route A so the kernel tail is a short reduce, not a copy+tree chain.
ROUTE_A_EXTRA = {61, 63}

_CACHE = {}


def _build_nc():
    nc = bacc.Bacc("TRN2", target_bir_lowering=False, debug=False,
                   num_devices=N_CORES)
    qT = nc.dram_tensor("qT", [PAIRS, 128, 128], F16, kind="ExternalInput").ap()
    dT = nc.dram_tensor("dT", [128, BD * SD], F16, kind="ExternalInput").ap()
    ones = nc.dram_tensor("ones", [128, Q_PER_CORE], F32, kind="ExternalInput").ap()
    # [2, PAIRS*BD]: row = pair member (query 2p / 2p+1), cols pair-major
    scores_out = nc.dram_tensor("scores", [2, PAIRS * BD], F32,
                                kind="ExternalOutput").ap()

    with tile.TileContext(nc) as tc:
        with (
            tc.tile_pool(name="qpool", bufs=1) as qpool,
            tc.tile_pool(name="docs", bufs=1) as dpool,
            tc.tile_pool(name="psum", bufs=2, space="PSUM") as pspool,
            tc.tile_pool(name="bcopy", bufs=3) as bpool,
            tc.tile_pool(name="scratch", bufs=2) as spool,
            tc.tile_pool(name="maxs", bufs=1) as mpool,
            tc.tile_pool(name="small", bufs=1) as smallpool,
        ):
            maxs = mpool.tile([128, PAIRS * BD], F32)

            # First doc chunk + queries first so matmuls can start ASAP.
            dtiles = [dpool.tile([128, DOCS_PER_GROUP * SD], F16,
                                 name=f"d{g}", tag=f"d{g}")
                      for g in range(GROUPS)]
            nc.sync.dma_start(dtiles[0][:], dT[:, 0:DOCS_PER_GROUP * SD])
            qtile = qpool.tile([128, PAIRS * 128], F16)
            nc.sync.dma_start(
                qtile[:].rearrange("q (p t) -> q p t", p=PAIRS),
                qT[:].rearrange("p q t -> q p t"))
            for g in range(1, GROUPS):
                nc.sync.dma_start(
                    dtiles[g][:],
                    dT[:, g * DOCS_PER_GROUP * SD:(g + 1) * DOCS_PER_GROUP * SD])
            # only needed by the final score matmuls
            otile = smallpool.tile([128, Q_PER_CORE], F32, tag="ones")
            nc.sync.dma_start(otile[:], ones[:])

            # maxs viewed as [128, pair, doc] for strided pair-tree outputs
            maxs_v = maxs[:].rearrange("q (p b) -> q p b", b=BD)

            def tt_max(out, in0, in1):
                nc.vector.tensor_tensor(out, in0, in1, op=mybir.AluOpType.max)

            def halves(ap, n):
                v = ap.rearrange("q (d h n) -> q d h n", h=2, n=n)
                return v[:, :, 0], v[:, :, 1]

            w = DOCS_PER_GROUP * SD
            pend = None                    # (sb_tile, (g, p)) of pending B half

            def emit_tree(sb, u0, u1):
                nd = 2 * DOCS_PER_GROUP    # 8 docs in the pair
                t1 = spool.tile([128, nd * 256], F16, name="t1", tag="t1")
                tt_max(t1[:].rearrange("q (d n) -> q d n", n=256),
                       *halves(sb[:], 256))
                t2 = spool.tile([128, nd * 128], F16, name="t2", tag="t2")
                tt_max(t2[:].rearrange("q (d n) -> q d n", n=128),
                       *halves(t1[:], 128))
                t3 = spool.tile([128, nd * 64], F16, name="t3", tag="t3")
                tt_max(t3[:].rearrange("q (d n) -> q d n", n=64),
                       *halves(t2[:], 64))
                (g0, p0), (g1, p1) = u0, u1
                if g0 == g1 and p1 == p0 + 1:
                    nc.vector.tensor_reduce(
                        maxs_v[:, p0:p0 + 2,
                               g0 * DOCS_PER_GROUP:(g0 + 1) * DOCS_PER_GROUP],
                        t3[:].rearrange("q (u d n) -> q u d n", u=2, n=64),
                        axis=mybir.AxisListType.X, op=mybir.AluOpType.max)
                else:
                    for h, (gh, ph) in enumerate(
                            ((g0, p0), (g1, p1))):
                        nc.vector.tensor_reduce(
                            maxs_v[:, ph:ph + 1,
                                   gh * DOCS_PER_GROUP:(gh + 1) * DOCS_PER_GROUP],
                            t3[:, h * DOCS_PER_GROUP * 64:(h + 1) * DOCS_PER_GROUP * 64]
                            .rearrange("q (d n) -> q d n", n=64),
                            axis=mybir.AxisListType.X, op=mybir.AluOpType.max)

            for g in range(GROUPS):
                for p in range(PAIRS):
                    unit = g * PAIRS + p
                    ps = pspool.tile([128, DOCS_PER_GROUP * SD], F32,
                                     name="ps", tag="ps")
                    for j in range(DOCS_PER_GROUP):
                        nc.tensor.matmul(
                            ps[:, j * SD:(j + 1) * SD],
                            lhsT=qtile[:, p * 128:(p + 1) * 128],
                            rhs=dtiles[g][:, j * SD:(j + 1) * SD],
                            start=True, stop=True,
                        )
                    col = p * BD + g * DOCS_PER_GROUP
                    if unit % 16 in ROUTE_A_MOD16 or unit in ROUTE_A_EXTRA:
                        nc.vector.tensor_reduce(
                            maxs[:, col:col + DOCS_PER_GROUP],
                            ps[:].rearrange("q (d n) -> q d n", n=SD),
                            axis=mybir.AxisListType.X,
                            op=mybir.AluOpType.max,
                        )
                        continue
                    # Route B: ACT copies this unit's 4 docs into one half of a
                    # double-wide bf16 tile; when the second half lands, one
                    # shared TT-max tree (2x_1p mode) drains all 8 docs.
                    if pend is None:
                        sb = bpool.tile([128, 2 * w], F16, name="b", tag="b")
                        nc.scalar.copy(sb[:, 0:w], ps[:])
                        pend = (sb, (g, p))
                    else:
                        sb, u0 = pend
                        nc.scalar.copy(sb[:, w:2 * w], ps[:])
                        emit_tree(sb, u0, (g, p))
                        pend = None
            assert pend is None, "odd number of route-B units"

            # scores[2, 64] per pair = (INV_TEMP * qmask)-weighted column sums
            scores_sb = smallpool.tile([2, PAIRS * BD], F32, tag="ssb")
            for p in range(PAIRS):
                sc_ps = pspool.tile([128, DOCS_PER_GROUP * SD], F32, tag="ps")
                nc.tensor.matmul(
                    sc_ps[0:2, 0:BD],
                    lhsT=otile[:, 2 * p:2 * p + 2],
                    rhs=maxs[:, p * BD:(p + 1) * BD],
                    start=True, stop=True,
                )
                nc.vector.tensor_copy(scores_sb[0:2, p * BD:(p + 1) * BD],
                                      sc_ps[0:2, 0:BD])
            nc.sync.dma_start(scores_out[:], scores_sb[:])

    nc.compile()
    return nc


def _get_nc():
    if "nc" not in _CACHE:
        _CACHE["nc"] = _build_nc()
    return _CACHE["nc"]


def _compact_doc_tokens(doc, mask):
    """Reorder each doc's tokens so masked slots are replaced by duplicates of
    a valid token (max over tokens is unchanged). Exact for any doc with at
    least one valid token."""
    out = doc.copy()
    for i in range(doc.shape[0]):
        m = mask[i]
        if m.all():
            continue
        valid = np.where(m)[0]
        idx = np.where(m, np.arange(doc.shape[1]), valid[0])
        out[i] = doc[i, idx]
    return out


def _host_reference(query_embeddings, doc_embeddings, query_mask, doc_mask):
    """Exact (fp32-semantics) fallback, only used for degenerate masks."""
    q = np.asarray(query_embeddings, np.float32)
    d = np.asarray(doc_embeddings, np.float32)
    sim = np.einsum('qnd,pmd->qpnm', q, d).astype(np.float32)
    sim = np.where(np.asarray(doc_mask, bool)[None, :, None, :], sim,
                   np.float32(-1e30))
    mx = sim.max(axis=-1)
    mx = mx * np.asarray(query_mask, np.float32)[:, None, :]
    scores = mx.sum(axis=-1) / np.float32(0.02)
    return _loss_from_scores(scores)


def _loss_from_scores(scores):
    s = np.asarray(scores, np.float64)
    m = s.max(axis=-1, keepdims=True)
    lse = m[:, 0] + np.log(np.exp(s - m).sum(axis=-1))
    return np.float32(np.mean(lse - np.diagonal(s)))


def kernel(query_embeddings, doc_embeddings, query_mask, doc_mask):
    q = np.ascontiguousarray(np.asarray(query_embeddings, dtype=np.float32))
    d = np.ascontiguousarray(np.asarray(doc_embeddings, dtype=np.float32))
    qm = np.asarray(query_mask, dtype=bool)
    dm = np.asarray(doc_mask, dtype=bool)
    assert q.shape == (BQ, SQ, D) and d.shape == (BD, SD, D)

    if not dm.all():
        if not dm.any(axis=1).all():
            # A fully-masked doc makes every max -1e30; the kernel's
            # compaction trick can't represent that, fall back entirely.
            return _host_reference(q, d, qm, dm)
        d = _compact_doc_tokens(d, dm)

    # [128(D), 64*512] doc tokens, bf16
    dT = np.ascontiguousarray(d.transpose(2, 0, 1).reshape(D, BD * SD)).astype(F16_NP)

    qmf = qm.astype(np.float32) * INV_TEMP
    in_maps = []
    for c in range(N_CORES):
        qc = q[c * Q_PER_CORE:(c + 1) * Q_PER_CORE]          # [8, 64, 128]
        # [pair, D, 128 tokens] bf16
        qT = np.ascontiguousarray(
            qc.reshape(PAIRS, 2 * SQ, D).transpose(0, 2, 1)).astype(F16_NP)
        ones = np.zeros((128, Q_PER_CORE), np.float32)
        for j in range(Q_PER_CORE):
            p, mzz = j // 2, j % 2
            ones[mzz * SQ:(mzz + 1) * SQ, j] = qmf[c * Q_PER_CORE + 2 * p + mzz]
        in_maps.append({"qT": qT, "dT": dT, "ones": ones})

    nc = _get_nc()
    res = run_bass_kernel_spmd(nc, in_maps, list(range(N_CORES)))
    scores = np.concatenate(
        [res.results[c]["scores"].reshape(2, PAIRS, BD).transpose(1, 0, 2)
         .reshape(Q_PER_CORE, BD) for c in range(N_CORES)], axis=0)  # [64, 64]
    return _loss_from_scores(scores)


if __name__ == "__main__":
    rng = np.random.default_rng(0)
    inputs = {
        "query_embeddings": rng.standard_normal((BQ, SQ, D), dtype=np.float32),
        "doc_embeddings": rng.standard_normal((BD, SD, D), dtype=np.float32),
        "query_mask": np.ones((BQ, SQ), bool),
        "doc_mask": np.ones((BD, SD), bool),
    }
    out = kernel(**inputs)
    ref = _host_reference(**inputs)
    print("kernel:", out, "ref:", ref, "rel:", abs(out - ref) / abs(ref))
